# revision 25
# baseline (speedup 1.0000x reference)
"""MiniAttention Trainium2 Bass kernel (v2.2 — transpose-free mix2, fp8
score path, merged evacuations, overlapped proj).

Problem: B=8, N=1024, C=768, H=12, D=64.
  qkv = x @ w_qkv.T ; q,k,v heads ; S = (q*SCALE) @ k.T per head
  A1 = conv_l-mix over heads ; P = softmax_m(A1) ; A2 = conv_w-mix of P
  out = (A2 @ v per head) @ w_proj.T + b_proj

Sharding: pure batch-parallel, 1 batch element per NeuronCore (8 cores).

Per-core design:
  - QKV: host packs w_qkv^T as [p, ct, kc, 128] so each 128-cout chunk
    loads contiguously; loads are issued per-chunk and pipeline under
    the q,k matmuls. q,k land [cout, n]; v lands [n, cout].
  - Scores per head: one 2-bank PSUM tile [128, 2, 512]; heads 2t/2t+1
    sit on partitions 0:64/64:128 of qk tile t so their matmuls row-tile
    and run concurrently. ONE evacuation per head (f32 -> fp8 e4m3,
    numerically validated) into sphall [nb, 12, 2, 512].
  - Head-interleave fan-out DMA per query group (src [gs, 12, 2, 512]
    -> dst [12*gs, ...] pairing rows p = r*12 + h), split across the
    gpsimd/sync/scalar DMA queues. fp8 halves the fabric bytes.
  - mix1: 120x120 matmul, lhsT = m1w in fp8 scaled by 32 (small conv_l
    values would hit e4m3 subnormals); exp applies scale=1/32 on ACT
    with accum_out row sums; reciprocal + conv_w scaling (w2) on DVE,
    pipelined one group later so no engine head-of-line blocking.
  - mix2 TRANSPOSED (no xbar DMA transpose): per 128-wide m-chunk,
    matmul(out[m-chunk, rows], lhsT=P chunk, rhs=w2) puts A2^T straight
    into PSUM; evac'd bf16 into aT [128, 12 g, 8 c, 120] which is
    exactly what attention@V streams.
  - attention@V: lhsT = V chunks, rhs = aT slices; head pairs col-tile
    into partition halves (explicit tile_position) and j-pairs share a
    2-bank PSUM tile so 6 couts evac in 3 copies.
  - proj: first n-half's chains emitted as soon as blocks 0-4 finish
    AV, overlapping the remaining attention blocks; second half after
    the last block.
"""

import numpy as np
import ml_dtypes

B, N, C, H = 8, 1024, 768, 12
D = C // H
SCALE = D ** -0.5
G = 10          # queries per mix group
NB = 120        # queries per block (12 groups)
NBLK = 8        # full blocks; last block is ragged: 6 groups of 10 + 1 of 4
M1SCALE = 32.0  # fp8 m1w prescale, undone by the exp input scale
BF16 = ml_dtypes.bfloat16
F8 = ml_dtypes.float8_e4m3

_cached = None


def _block_layout():
    """(n0, nb, chunks); chunks = (row_start, g_start, g_count, g_size)."""
    blocks = []
    for b in range(NBLK):
        blocks.append((b * NB, NB, [(0, 0, 12, G)]))
    blocks.append((960, 64, [(0, 0, 6, G), (60, 6, 1, 4)]))
    return blocks


def _build_program():
    import concourse.tile as tile
    from concourse import bacc, mybir

    f32 = mybir.dt.float32
    bf16 = mybir.dt.bfloat16
    f8 = mybir.dt.float8e4
    Exp = mybir.ActivationFunctionType.Exp

    nc = bacc.Bacc("TRN2", target_bir_lowering=False, debug=False)

    xt = nc.dram_tensor("xt", [C, N], bf16, kind="ExternalInput").ap()
    wqkvt = nc.dram_tensor("wqkvt", [128, 18 * 6 * 128], bf16,
                           kind="ExternalInput").ap()
    wprojt = nc.dram_tensor("wprojt", [C, C], bf16, kind="ExternalInput").ap()
    m1w_in = nc.dram_tensor("m1w", [120, 120], f8, kind="ExternalInput").ap()
    m1w4_in = nc.dram_tensor("m1w4", [48, 48], f8, kind="ExternalInput").ap()
    m2p_in = nc.dram_tensor("m2p", [120, 128], f32, kind="ExternalInput").ap()
    m2p4_in = nc.dram_tensor("m2p4", [48, 128], f32, kind="ExternalInput").ap()
    out_d = nc.dram_tensor("out", [C, N], f32, kind="ExternalOutput").ap()

    KC = C // 128  # 6 contraction chunks

    _ec = [0]

    def evac(dst, src):
        i = _ec[0]
        _ec[0] += 1
        if i % 8 < 3:
            nc.scalar.copy(dst, src)
        else:
            nc.vector.tensor_copy(dst, src)

    with tile.TileContext(nc) as tc:
        with tc.tile_pool(name="const", bufs=1) as const, \
             tc.tile_pool(name="big", bufs=1) as big:

            m1wsb = const.tile([120, 120], f8)
            nc.gpsimd.dma_start(m1wsb, m1w_in)
            m1w4sb = const.tile([48, 48], f8)
            nc.gpsimd.dma_start(m1w4sb, m1w4_in)
            m2psb = const.tile([120, 128], f32)
            nc.gpsimd.dma_start(m2psb, m2p_in)
            m2p4sb = const.tile([48, 128], f32)
            nc.gpsimd.dma_start(m2p4sb, m2p4_in)
            wpsb = const.tile([128, KC, C], bf16)

            # persistent activations
            qksb = big.tile([128, 2 * KC, N], bf16)   # ct 0..5 = q, 6..11 = k
            vsb = big.tile([128, 8, C], bf16)         # [m%128, m//128, cout]
            attnT = big.tile([128, KC, N], bf16)      # [cout2%128, ., n]

            # ---- one scope: QKV + attention + proj (shared PSUM) ----
            with tc.tile_pool(name="xtp", bufs=1) as xtp, \
                 tc.tile_pool(name="spl", bufs=2) as spl, \
                 tc.tile_pool(name="sint", bufs=2) as sintp, \
                 tc.tile_pool(name="pgp", bufs=4) as pgp, \
                 tc.tile_pool(name="aTp", bufs=2) as aTp, \
                 tc.tile_pool(name="smp", bufs=5) as smp, \
                 tc.tile_pool(name="m2wp", bufs=5) as m2wp, \
                 tc.tile_pool(name="outp", bufs=2) as outp, \
                 tc.tile_pool(name="ps1k", bufs=3, space="PSUM") as ps1k, \
                 tc.tile_pool(name="psA2", bufs=2, space="PSUM") as psA2:

                xtsb = xtp.tile([128, KC, N], bf16)
                xt_r = xt.rearrange("(kc p) n -> p kc n", p=128)
                # 3-way split: the first qk chains start as soon as their
                # early kc chunks land instead of waiting ~11us for one
                # serial queue to deliver all of xt
                nc.scalar.dma_start(xtsb[:, 0:2, :], xt_r[:, 0:2, :])
                nc.sync.dma_start(xtsb[:, 2:4, :], xt_r[:, 2:4, :])
                nc.gpsimd.dma_start(xtsb[:, 4:6, :], xt_r[:, 4:6, :])
                wqsb = xtp.tile([128, 18, KC, 128], bf16)
                wq_r = wqkvt.rearrange("p (ct kc c) -> p ct kc c",
                                       ct=18, kc=KC)
                for ct in range(18):
                    eng = nc.sync if ct % 2 == 0 else nc.gpsimd
                    eng.dma_start(wqsb[:, ct, :, :], wq_r[:, ct, :, :])
                nc.sync.dma_start(
                    wpsb, wprojt.rearrange("(kc p) c -> p kc c", p=128))

                def emit_qk():
                    for ct in range(12):
                        for nh in range(2):
                            ps = ps1k.tile([128, 2, 512], f32, tag="ps1k",
                                           name="qkv_ps")
                            for kc in range(KC):
                                nc.tensor.matmul(
                                    ps[:, nh, :],
                                    lhsT=wqsb[:, ct, kc, :],
                                    rhs=xtsb[:, kc, 512 * nh:512 * nh + 512],
                                    start=(kc == 0), stop=(kc == KC - 1),
                                )
                            evac(qksb[:, ct, 512 * nh:512 * nh + 512],
                                 ps[:, nh, :])

                def emit_v():
                    for nt in range(8):
                        ps = ps1k.tile([128, 2, 512], f32, tag="ps1k",
                                       name="v_ps")
                        for (ct0, ct1, mh, c0) in [(12, 16, 0, 0),
                                                   (16, 18, 1, 0)]:
                            for kc in range(KC):
                                nc.tensor.matmul(
                                    ps[:, mh, c0:c0 + 128 * (ct1 - ct0)],
                                    lhsT=xtsb[:, kc,
                                              128 * nt:128 * nt + 128],
                                    rhs=wqsb[:, ct0:ct1, kc, :],
                                    start=(kc == 0), stop=(kc == KC - 1),
                                )
                        evac(vsb[:, nt, 0:512], ps[:, 0, :])
                        evac(vsb[:, nt, 512:768], ps[:, 1, 0:256])

                _sc = [0]
                _pc = [0]

                def emit_score_head(n0, nb, h):
                    """Two score matmuls into one 2-bank tile + ONE fp8
                    evacuation per head."""
                    sphall, sint = cur_sc
                    base = 64 * (h % 2)
                    ps = ps1k.tile([128, 2, 512], f32, tag="ps1k",
                                   name="ps1k")
                    for mh in range(2):
                        nc.tensor.matmul(
                            ps[0:nb, mh, :],
                            lhsT=qksb[base:base + 64, h // 2, n0:n0 + nb],
                            rhs=qksb[base:base + 64, 6 + h // 2,
                                     512 * mh:512 * mh + 512],
                            start=True, stop=True,
                        )
                    i = _sc[0]
                    _sc[0] += 1
                    if i % 3 == 0:
                        nc.scalar.copy(sphall[0:nb, h, :, :], ps[0:nb, :, :])
                    else:
                        nc.vector.tensor_copy(
                            sphall[0:nb, h, :, :], ps[0:nb, :, :])

                def emit_interleave(chunks):
                    """Fan-out interleave (row r*12+h <- (r, h)) split over
                    three DMA queues."""
                    sphall, sint = cur_sc
                    qi = 0
                    for (rs, g0, gc, gs) in chunks:
                        for gi in range(gc):
                            eng = (nc.gpsimd, nc.sync, nc.gpsimd)[qi % 3]
                            qi += 1
                            eng.dma_start(
                                out=sint[0:12 * gs, g0 + gi, :, :],
                                in_=sphall[rs + gi * gs:rs + (gi + 1) * gs,
                                           :, :, :],
                            )

                def emit_pass1a_group(st, rs_g, g, gs):
                    """mix1 (fp8, prescaled) -> exp(x/32) + row sums."""
                    rows = 12 * gs
                    m1 = m1wsb if gs == G else m1w4sb
                    sm = smp.tile([128, 4], f32, tag="sm")
                    pg = pgp.tile([128, 2, 512], bf16, tag="pg")
                    a1 = ps1k.tile([128, 2, 512], f32, tag="ps1k",
                                   name="a1")
                    for mh in range(2):
                        nc.tensor.matmul(
                            a1[0:rows, mh, :],
                            lhsT=m1,
                            rhs=st["sint"][0:rows, g, mh, :],
                            start=True, stop=True,
                        )
                    nc.scalar.activation(
                        pg[0:rows, :, :], a1[0:rows, :, :], Exp,
                        scale=1.0 / M1SCALE,
                        accum_out=sm[0:rows, 0:1],
                    )
                    st["pgs"][g] = pg
                    st["sms"][g] = sm

                def emit_pass1b_group(st, rs_g, g, gs):
                    """1/sum -> w2 (scaled conv_w rhs), one step later."""
                    rows = 12 * gs
                    m2 = m2psb if gs == G else m2p4sb
                    sm = st["sms"][g]
                    nc.vector.reciprocal(sm[0:rows, 3:4], sm[0:rows, 0:1])
                    w2 = m2wp.tile([128, 128], bf16, tag="m2w")
                    nc.vector.tensor_scalar_mul(
                        w2[0:rows, :], m2, sm[0:rows, 3:4])
                    st["w2s"][g] = (w2, rows)

                def emit_pass2_group(st, g):
                    """Transposed mix2: A2^T per m-chunk; evac bf16 to aT."""
                    pg = st["pgs"][g]
                    w2, rows = st["w2s"][g]
                    aTt = st["aT"]
                    for half in range(2):
                        a2 = psA2.tile([128, 4, 128], f32, tag="psA2",
                                       name="a2")
                        for ci in range(4):
                            c = 4 * half + ci
                            nc.tensor.matmul(
                                a2[:, ci, 0:rows],
                                lhsT=pg[0:rows, c // 4,
                                        128 * (c % 4):128 * (c % 4) + 128],
                                rhs=w2[0:rows, 0:rows],
                                start=True, stop=True,
                            )
                        i = _pc[0]
                        _pc[0] += 1
                        if i % 5 < 2:
                            nc.scalar.copy(
                                aTt[:, g, 4 * half:4 * half + 4, 0:rows],
                                a2[:, :, 0:rows])
                        else:
                            nc.vector.tensor_copy(
                                aTt[:, g, 4 * half:4 * half + 4, 0:rows],
                                a2[:, :, 0:rows])

                def emit_av(st):
                    """Attention@V; head pairs col-tile into partition
                    halves, j-pairs share a 2-bank tile (3 evacs).
                    NOTE: ragged chunks stay OUTER relative to the c-chain
                    (has_written start-clear covers the instruction's
                    partitions bank-wide)."""
                    n0, nb, chunks = st["n0"], st["nb"], st["chunks"]
                    aTt = st["aT"]
                    for jp in range(3):
                        av = ps1k.tile([128, 2, 512], f32, tag="ps1k",
                                       name="av")
                        for jj in range(2):
                            j = 2 * jp + jj
                            for (rs, g0, gc, gs) in chunks:
                                for c in range(8):
                                    for oo in (2 * j, 2 * j + 1):
                                        p0 = 64 * (oo & 1)
                                        nc.tensor.matmul(
                                            av[p0:p0 + 64, jj,
                                               rs:rs + gc * gs],
                                            lhsT=vsb[:, c,
                                                     64 * oo:64 * oo + 64],
                                            rhs=aTt[:, g0:g0 + gc, c,
                                                    gs * oo:gs * oo + gs],
                                            start=(c == 0), stop=(c == 7),
                                            tile_position=(0, p0),
                                        )
                        if jp % 2 == 0:
                            nc.scalar.copy(
                                attnT[:, 2 * jp:2 * jp + 2, n0:n0 + nb],
                                av[:, :, 0:nb])
                        else:
                            nc.vector.tensor_copy(
                                attnT[:, 2 * jp:2 * jp + 2, n0:n0 + nb],
                                av[:, :, 0:nb])

                od = out_d.rearrange("(ct p) n -> p ct n", p=128)

                def emit_proj_chain(ct, c0, ncols=512):
                    ps = ps1k.tile([128, 2, 512], f32, tag="ps1k",
                                   name="pj")
                    for kc in range(KC):
                        nc.tensor.matmul(
                            ps[:, 0, 0:ncols],
                            lhsT=wpsb[:, kc, 128 * ct:128 * ct + 128],
                            rhs=attnT[:, kc, c0:c0 + ncols],
                            start=(kc == 0), stop=(kc == KC - 1),
                        )
                    ob = outp.tile([128, 512], f32, tag="ob")
                    evac(ob[:, 0:ncols], ps[:, 0, 0:ncols])
                    eng = nc.sync if ct % 2 == 0 else nc.gpsimd
                    eng.dma_start(od[:, ct, c0:c0 + ncols], ob[:, 0:ncols])

                def groups_of(chunks):
                    return [(rs + gi * gs, g0 + gi, gs)
                            for (rs, g0, gc, gs) in chunks
                            for gi in range(gc)]

                def emit_block_body(bst, score_heads, score_n0, score_nb):
                    """3-stage pipelined group loop: pass1a(g) at step g,
                    pass1b(g) at g+1, pass2(g) at g+2; next block's score
                    heads two per step so the interleave starts early."""
                    glist = groups_of(bst["chunks"]) if bst else []
                    heads = list(score_heads)
                    p1b = []
                    p2 = []
                    steps = max(len(glist) + 2, (len(heads) + 1) // 2)
                    for i in range(steps):
                        for _ in range(2):
                            if heads:
                                emit_score_head(score_n0, score_nb,
                                                heads.pop(0))
                        if i < len(glist):
                            emit_pass1a_group(bst, *glist[i])
                            p1b.append(glist[i])
                        if len(p1b) > (1 if i < len(glist) else 0):
                            g = p1b.pop(0)
                            emit_pass1b_group(bst, *g)
                            p2.append(g)
                        if len(p2) > (1 if p1b else 0):
                            emit_pass2_group(bst, p2.pop(0)[1])
                    while p2:
                        emit_pass2_group(bst, p2.pop(0)[1])

                # ---- main pipelined loop over blocks ----
                emit_qk()
                prev = None
                prev2 = None
                cur_sc = None
                av_done = 0
                proj_next = 0
                first = True
                for (n0, nb, chunks) in _block_layout():
                    sphall_t = spl.tile([128, 12, 2, 512], f8, tag="spl",
                                        name="sphall")
                    sint_t = sintp.tile([128, 12, 2, 512], f8, tag="sint",
                                        name="sint")
                    cur_sc = (sphall_t, sint_t)
                    if first:
                        # block 0 scores + interleave overlap the v matmuls
                        for h in range(12):
                            emit_score_head(n0, nb, h)
                        emit_interleave(chunks)
                        emit_v()
                        prev = {"n0": n0, "nb": nb, "chunks": chunks,
                                "sint": sint_t}
                        first = False
                        continue
                    if prev is not None:
                        prev["aT"] = aTp.tile([128, 12, 8, 120], bf16,
                                              tag="aT", name="aT")
                        prev["pgs"] = {}
                        prev["w2s"] = {}
                        prev["sms"] = {}
                    if prev2 is not None:
                        emit_av(prev2)
                        av_done += 1
                        # first output n-half overlaps remaining blocks
                        if av_done >= 5:
                            for _ in range(2):
                                if proj_next < 6:
                                    emit_proj_chain(proj_next, 0)
                                    proj_next += 1
                    emit_block_body(prev, list(range(12)), n0, nb)
                    emit_interleave(chunks)
                    prev2 = prev
                    prev = {"n0": n0, "nb": nb, "chunks": chunks,
                            "sint": sint_t}
                # drain
                prev["aT"] = aTp.tile([128, 12, 8, 120], bf16, tag="aT",
                                      name="aT")
                prev["pgs"] = {}
                prev["w2s"] = {}
                prev["sms"] = {}
                if prev2 is not None:
                    emit_av(prev2)
                while proj_next < 6:
                    emit_proj_chain(proj_next, 0)
                    proj_next += 1
                # cols 512:960 only need blocks 4-7: overlap the tail
                # block's group loop (which waits on its interleave DMA);
                # only cols 960:1024 truly depend on the tail block's AV
                for ct in range(KC):
                    emit_proj_chain(ct, 512, 256)
                for ct in range(KC):
                    emit_proj_chain(ct, 768, 192)
                emit_block_body(prev, [], 0, 0)
                emit_av(prev)
                for ct in range(KC):
                    emit_proj_chain(ct, 960, 64)

    nc.compile()
    return nc


def _mix_weights(conv_l_w, conv_w_w):
    """Host-built mix matrices.

    sint rows are r-major interleaved: p = r*12 + h. A1/pg rows o-major:
    c = o*gs + r.

    m1w[r*12+h, o*gs+r2] = (r==r2) * M1SCALE * SCALE * conv_l[o, h]
        (fp8 lhsT for mix1; the 32x prescale keeps values out of e4m3
        subnormals, undone by exp's input scale)
    m2p[o1*gs+r, o2*gs+r2] = (r==r2) * conv_w[o2, o1]  (f32 pattern;
        scaled per-group by 1/softmax_sum; streaming rhs of mix2T)
    """
    outs = []
    for gs in (G, 4):
        rows = 12 * gs
        m1 = np.zeros((rows, rows), np.float32)
        m2 = np.zeros((rows, 128), np.float32)
        for h in range(12):
            for o in range(12):
                for r in range(gs):
                    m1[r * 12 + h, o * gs + r] = \
                        M1SCALE * SCALE * conv_l_w[o, h]
                    m2[h * gs + r, o * gs + r] = conv_w_w[o, h]
        outs.append((m1.astype(F8), m2.astype(np.float32)))
    (m1w, m2p), (m1w4, m2p4) = outs
    return m1w, m2p, m1w4, m2p4


def _run(x, w_qkv, w_proj, b_proj, conv_l_w, conv_w_w, **spmd_kwargs):
    global _cached
    from concourse import bass_utils

    x = np.asarray(x, np.float32)
    w_qkv = np.asarray(w_qkv, np.float32)
    w_proj = np.asarray(w_proj, np.float32)
    b_proj = np.asarray(b_proj, np.float32)
    conv_l_w = np.asarray(conv_l_w, np.float32)
    conv_w_w = np.asarray(conv_w_w, np.float32)

    if _cached is None:
        _cached = _build_program()
    nc = _cached

    m1w, m2p, m1w4, m2p4 = _mix_weights(conv_l_w, conv_w_w)
    # wqkvt packed [p, ct, kc, 128] so per-ct chunks load contiguously
    wq = np.ascontiguousarray(w_qkv.T).astype(BF16)          # [C, 3C]
    wq4 = wq.reshape(6, 128, 18, 128).transpose(1, 2, 0, 3)  # [p,ct,kc,c]
    wqkvt = np.ascontiguousarray(wq4).reshape(128, 18 * 6 * 128)
    wprojt = np.ascontiguousarray(w_proj.T).astype(BF16)

    in_maps = []
    for b in range(B):
        in_maps.append({
            "xt": np.ascontiguousarray(x[b].T).astype(BF16),
            "wqkvt": wqkvt,
            "wprojt": wprojt,
            "m1w": m1w,
            "m1w4": m1w4,
            "m2p": m2p,
            "m2p4": m2p4,
        })

    res = bass_utils.run_bass_kernel_spmd(
        nc, in_maps, core_ids=list(range(B)), **spmd_kwargs)
    out = np.stack([res.results[b]["out"].T for b in range(B)])  # [B, N, C]
    return (out + b_proj[None, None, :]).astype(np.float32), res


def kernel(x, w_qkv, w_proj, b_proj, conv_l_w, conv_w_w):
    out, _ = _run(x, w_qkv, w_proj, b_proj, conv_l_w, conv_w_w)
    return out
